# revision 52
# baseline (speedup 1.0000x reference)
"""Multi-head causal attention (B=2, S=2048, D=1024, H=16, HD=64) on 8 TRN2 cores.

Sharding: core c handles batch b = c//4 and heads 4*(c%4)..4*(c%4)+3.
The reference reshapes [b,h,s,hd] -> [b,s,1024] WITHOUT head transpose-back,
so output rows [128h, 128h+128) of y[b] depend only on head h: each core
produces a disjoint [512, 1024] block of the output. No collectives.

v3 (bf16 + trim + skewed pipeline + phase interleave):
  - All weight/activation DRAM inputs in bf16 (host-converted).
  - Diagonal score tiles trimmed: scores matmul, exp, and PV only cover
    q-cols >= 128r of the 512-q block; the partial 128x128 triangle block is
    masked post-exp by one strided bf16 DVE multiply (both heads at once).
  - Attention inner loop is software-pipelined: scores(t+1) is emitted
    before PV(t) so the in-order PE queue never serializes behind exp(t).
  - Pair-1 Q/K projections and pair-0 normalize/projection are emitted as
    fillers inside the opposite pair's attention j-loop, placed right after
    the second scores emission (where PE would otherwise stall on exp).
  - Normalization is per-j: denominators ride the PV matmul as a 65th V
    column, are DMA-gathered into dall rows (2j+q), reciprocal'd per j-block,
    broadcast via one-hot matmul, applied by DVE/Pool multiplies.
  - Output projection at K=128: attnT2b[h] is [128, 2048] with partitions
    64:128 holding a 1-col-left-shifted copy of rows 0:64 (SBUF->SBUF DMA),
    so lhsT [128,128] packs head-chunk pairs (m, m+1) and Wo contracts in 8
    chunks of 128 instead of 16 of 64 (halves proj PE rows).
"""

import sys

if "/opt/trn_rl_repo" not in sys.path:
    sys.path.insert(0, "/opt/trn_rl_repo")

from contextlib import ExitStack

import numpy as np
import ml_dtypes

import concourse.bass as bass
import concourse.tile as tile
from concourse import bacc, mybir

F32 = mybir.dt.float32
F32R = mybir.dt.float32r
BF16 = mybir.dt.bfloat16
EXP = mybir.ActivationFunctionType.Exp

B, S, D, H, HD = 2, 2048, 1024, 16, 64
NC = 8
HPC = 4  # heads per core
CT = D // 128  # 8 contraction tiles
QB = 4  # q-blocks of 512
KT = S // 128  # 16 k-tiles
SCALE = 1.0 / 8.0
NPBF16 = ml_dtypes.bfloat16


def build_nc():
    nc = bacc.Bacc("TRN2", target_bir_lowering=False, debug=False)

    xt = nc.dram_tensor("xt", [128, CT, S], BF16, kind="ExternalInput").ap()
    wq = nc.dram_tensor("wq", [128, 2, CT, 128], BF16, kind="ExternalInput").ap()
    wk = nc.dram_tensor("wk", [128, 2, CT, 128], BF16, kind="ExternalInput").ap()
    wv = nc.dram_tensor("wv", [128, CT, 256], BF16, kind="ExternalInput").ap()
    wo2 = nc.dram_tensor("wo2", [128, 8, 4, 256], BF16, kind="ExternalInput").ap()
    bo = nc.dram_tensor("bo", [D], BF16, kind="ExternalInput").ap()
    masks = nc.dram_tensor("masks", [128, 2, 128], BF16, kind="ExternalInput").ap()
    oneh = nc.dram_tensor("oneh", [2, 128], BF16, kind="ExternalInput").ap()
    y = nc.dram_tensor("y", [HPC * 128, D], F32, kind="ExternalOutput").ap()

    with tile.TileContext(nc) as tc, ExitStack() as ctx:
        a_pool = ctx.enter_context(tc.tile_pool(name="a", bufs=1))

        # ---- resident SBUF tensors
        xt_sb = a_pool.tile([128, CT, S], BF16, tag="xt")
        wq_sb = a_pool.tile([128, 2, CT, 128], BF16, tag="wq")
        wk_sb = a_pool.tile([128, 2, CT, 128], BF16, tag="wk")
        wv_sb = a_pool.tile([128, CT, 256], BF16, tag="wv")
        wo2_sb = a_pool.tile([128, 8, 4, 256], BF16, tag="wo2")
        masks_sb = a_pool.tile([128, 2, 128], BF16, tag="masks")
        oneh_sb = a_pool.tile([2, 128], BF16, tag="oneh")
        bo_sb = a_pool.tile([128, D], BF16, tag="bo")
        # V packed [128(s_local), 16 s-tiles, 4*(64+ones col)] bf16
        v4 = a_pool.tile([128, KT, 260], BF16, tag="v4")
        qst = [a_pool.tile([128, S], BF16, tag=f"qst{p}", name=f"qst{p}") for p in range(2)]
        kst = [a_pool.tile([128, S], BF16, tag=f"kst{p}", name=f"kst{p}") for p in range(2)]
        qtb = [a_pool.tile([64, S], BF16, tag=f"qtb{p}", name=f"qtb{p}") for p in range(2)]
        ktb = [a_pool.tile([64, S], BF16, tag=f"ktb{p}", name=f"ktb{p}") for p in range(2)]
        # attnT2b[h]: rows 0:64 = attn^T (hd x q), rows 64:128 = 1-col-left-
        # shifted copy (for K=128 proj lhsT)
        attnT2b = [
            a_pool.tile([128, S], BF16, tag=f"at{h}", name=f"at{h}") for h in range(HPC)
        ]
        # dall[p][j][q] = denominators of q-block j, head q of pair p
        # (separate [1,512] tiles: partition_broadcast needs base partition 0)
        dall = [
            [[a_pool.tile([1, 512], BF16, tag=f"dall{p}{j}{q}", name=f"dall{p}{j}{q}")
              for q in range(2)] for j in range(QB)]
            for p in range(2)
        ]
        dallr = [
            [[a_pool.tile([1, 512], BF16, tag=f"dallr{p}{j}{q}", name=f"dallr{p}{j}{q}")
              for q in range(2)] for j in range(QB)]
            for p in range(2)
        ]

        # warm up the Act engine's Exp table at t~0 (it otherwise lazy-loads
        # 1.3us right at the first attention exp)
        ones_col = a_pool.tile([1, 128], BF16, tag="ones_col")
        nc.vector.memset(ones_col[:], 1.0)
        warm = a_pool.tile([1, 8], F32, tag="warm")
        warm2 = a_pool.tile([1, 8], F32, tag="warm2")
        nc.vector.memset(warm[:], 0.0)
        nc.scalar.activation(warm2[:], warm[:], EXP, scale=SCALE)

        # ---- input DMAs (SP queue; order = need order; wo2 issued after P1)
        nc.sync.dma_start(out=xt_sb[:, 0, :], in_=xt[:, 0, :])
        nc.sync.dma_start(out=wq_sb[:, 0, 0], in_=wq[:, 0, 0])
        nc.sync.dma_start(out=wk_sb[:, 0, 0], in_=wk[:, 0, 0])
        nc.sync.dma_start(out=wq_sb[:, 0, 1:8], in_=wq[:, 0, 1:8])
        nc.sync.dma_start(out=wk_sb[:, 0, 1:8], in_=wk[:, 0, 1:8])
        nc.sync.dma_start(out=xt_sb[:, 1, :], in_=xt[:, 1, :])
        for quad in range(1, 4):
            nc.sync.dma_start(
                out=xt_sb[:, 2 * quad : 2 * quad + 2, :],
                in_=xt[:, 2 * quad : 2 * quad + 2, :],
            )
        nc.sync.dma_start(out=wv_sb[:], in_=wv)
        nc.sync.dma_start(out=wq_sb[:, 1], in_=wq[:, 1])
        nc.sync.dma_start(out=wk_sb[:, 1], in_=wk[:, 1])
        nc.sync.dma_start(out=masks_sb[:], in_=masks)
        nc.sync.dma_start(out=oneh_sb[:], in_=oneh)
        bo_b = bass.AP(tensor=bo.tensor, offset=bo.offset, ap=[[0, 128], [1, D]])
        nc.sync.dma_start(out=bo_sb[:], in_=bo_b)
        # ones column of v4 via memset (strided view)
        nc.gpsimd.memset(
            v4[:].rearrange("p t (h c) -> p t h c", c=65)[:, :, :, 64:65], 1.0
        )

        y_pool = ctx.enter_context(tc.tile_pool(name="y", bufs=6))
        rr_pool = ctx.enter_context(tc.tile_pool(name="rr", bufs=8))
        pt_pool = ctx.enter_context(tc.tile_pool(name="pt", bufs=3))

        # ---- P1 pair 0: Q/K ct-outer with 8 live psum accumulators, then V
        with ExitStack() as scope1:
            ps1 = scope1.enter_context(tc.tile_pool(name="ps1", bufs=2, space="PSUM"))
            psqk = [
                ps1.tile([128, 512], F32, tag=f"qk{i}", name=f"qk{i}", bufs=1)
                for i in range(8)
            ]
            for ct in range(CT):
                for i, w_sb in ((0, wq_sb), (4, wk_sb)):
                    for nb in range(QB):
                        nc.tensor.matmul(
                            psqk[i + nb][:],
                            w_sb[:, 0, ct, :],
                            xt_sb[:, ct, bass.ts(nb, 512)],
                            start=(ct == 0),
                            stop=(ct == CT - 1),
                        )
            for i, dst in ((0, qst[0]), (4, kst[0])):
                for nb in range(QB):
                    nc.vector.tensor_copy(dst[:, bass.ts(nb, 512)], psqk[i + nb][:])
            nc.sync.dma_start(out=qtb[0][:], in_=qst[0][64:128, :])
            nc.sync.dma_start(out=ktb[0][:], in_=kst[0][64:128, :])
            nc.sync.dma_start(out=wo2_sb[:], in_=wo2)
            # V for all 4 heads (st-outer, ct accumulation)
            for st in range(KT):
                ps = ps1.tile([128, 256], F32, tag=f"qk{st % 8}", name="psv", bufs=1)
                for ct in range(CT):
                    nc.tensor.matmul(
                        ps[:],
                        xt_sb[:, ct, bass.ts(st, 128)],
                        wv_sb[:, ct, :],
                        start=(ct == 0),
                        stop=(ct == CT - 1),
                    )
                nc.scalar.activation(
                    v4[:, st, :].rearrange("p (h c) -> p h c", c=65)[:, :, 0:64],
                    ps[:].rearrange("p (h c) -> p h c", c=64),
                    mybir.ActivationFunctionType.Copy,
                )

        # ---- P2: attention, software-pipelined, with interleaved fillers
        ps2 = ctx.enter_context(tc.tile_pool(name="ps2", bufs=2, space="PSUM"))

        def attn_scores(p, j, t, qv):
            """scores^T both heads -> exp -> mask (diagonal). Returns pt2."""
            r = t - 4 * j
            c0 = 128 * r if r > 0 else 0
            pss = ps2.tile([128, 1024], F32, tag="pss", name="pss", bufs=3)
            for q in range(2):
                qt, kt = qv[q]
                nc.tensor.matmul(
                    pss[:, 512 * q + c0 : 512 * (q + 1)],
                    kt[:, bass.ts(t, 128)],
                    qt[:, 512 * j + c0 : 512 * (j + 1)],
                    start=True,
                    stop=True,
                )
            pt2 = pt_pool.tile([128, 1024], BF16, tag="pt2", name="pt2")
            if r < 0:
                nc.scalar.activation(pt2[:], pss[:], EXP, scale=SCALE)
            else:
                pv = pss[:].rearrange("p (h c) -> p h c", c=512)[:, :, c0:512]
                ov = pt2[:].rearrange("p (h c) -> p h c", c=512)[:, :, c0:512]
                nc.scalar.activation(ov, pv, EXP, scale=SCALE)
                mv = pt2[:].rearrange("p (h c) -> p h c", c=512)[:, :, c0 : c0 + 128]
                nc.vector.tensor_mul(mv, mv, masks_sb[:])
            return pt2

        def attn_pv(p, j, t, pt2, psa):
            r = t - 4 * j
            c0 = 128 * r if r > 0 else 0
            last = t == 4 * j + 3
            for q in range(2):
                h = 2 * p + q
                vsl = v4[:, t, bass.ds(65 * h, 65)]
                if r >= 0:
                    # split stop: [c0, c0+128) is final here; rest continues
                    nc.tensor.matmul(
                        psa[q][0:65, c0 : c0 + 128],
                        vsl,
                        pt2[:, 512 * q + c0 : 512 * q + c0 + 128],
                        start=(t == 0),
                        stop=True,
                    )
                    if not last:
                        nc.tensor.matmul(
                            psa[q][0:65, c0 + 128 : 512],
                            vsl,
                            pt2[:, 512 * q + c0 + 128 : 512 * (q + 1)],
                            start=(t == 0),
                            stop=False,
                        )
                else:
                    nc.tensor.matmul(
                        psa[q][0:65, :],
                        vsl,
                        pt2[:, bass.ts(q, 512)],
                        start=(t == 0),
                        stop=False,
                    )

        def attn_j(p, j, qv, slot0=(), slot1=(), slot2=(), pre=None):
            """Skewed pipeline: S0 S1 [pre] [slot0] P0 S2 [slot1] P1 S3 ... PT.
            Each slot must stay under ~2us of PE work (the Act runway from
            the two queued exps) or Act starves behind the in-order PE queue.
            Returns a closure finishing this j (psa copies + denom gather),
            which the caller passes as `pre` to the NEXT attn_j so the next
            block's first scores are not queued behind it."""
            T = 4 * j + 4
            psa = [
                ps2.tile([128, 512], F32, tag=f"psa{q}", name=f"psa{q}", bufs=1)
                for q in range(2)
            ]
            pts = [attn_scores(p, j, 0, qv)]
            if T > 1:
                pts.append(attn_scores(p, j, 1, qv))
            if pre is not None:
                pre()
            for f in slot0:
                f()
            for t in range(T):
                if t + 2 < T:
                    pts.append(attn_scores(p, j, t + 2, qv))
                if t == 1:
                    for f in slot1:
                        f()
                if t == 5:
                    for f in slot2:
                        f()
                attn_pv(p, j, t, pts[t], psa)

            def finish():
                for q in range(2):
                    h = 2 * p + q
                    for half in range(2):
                        nc.vector.tensor_copy(
                            attnT2b[h][0:65, 512 * j + 256 * half : 512 * j + 256 * (half + 1)],
                            psa[q][0:65, bass.ts(half, 256)],
                        )
                    nc.sync.dma_start(
                        out=dall[p][j][q][:],
                        in_=attnT2b[h][64:65, bass.ts(j, 512)],
                    )

            return finish, psa

        def qk1_half(half, nb):
            """Pair-1 Q (half=0) or K (half=1) projection for one 512-col
            s-block (rides pss tag; ~1.7us of PE)."""
            w_sb, dst, dstb = ((wq_sb, qst[1], qtb[1]), (wk_sb, kst[1], ktb[1]))[half]
            pssqk = ps2.tile([128, 1024], F32, tag="pss", name="pssqk", bufs=3)
            for ct in range(CT):
                nc.tensor.matmul(
                    pssqk[:, 0:512],
                    w_sb[:, 1, ct, :],
                    xt_sb[:, ct, bass.ts(nb, 512)],
                    start=(ct == 0),
                    stop=(ct == CT - 1),
                )
            nc.vector.tensor_copy(dst[:, bass.ts(nb, 512)], pssqk[:, 0:512])
            nc.sync.dma_start(
                out=dstb[:, bass.ts(nb, 512)], in_=dst[64:128, bass.ts(nb, 512)]
            )

        def norm_j(p, cb):
            """Normalize q-block cb of both heads of pair p, then emit the
            shifted duplicate rows for the K=128 projection. The reciprocal
            rows are partition-broadcast by DMA (stride-0 source) so the
            multiplies are pure-SBUF bf16 (DVE 4x) and PE stays free."""
            for q in range(2):
                h = 2 * p + q
                with nc.allow_low_precision(reason="softmax denom reciprocal"):
                    nc.vector.reciprocal(dallr[p][cb][q][:], dall[p][cb][q][:])
                rr = rr_pool.tile([64, 512], BF16, tag="rr", name="rr")
                nc.gpsimd.partition_broadcast(rr[:], dallr[p][cb][q][:])
                nc.vector.tensor_mul(
                    attnT2b[h][0:64, bass.ts(cb, 512)],
                    attnT2b[h][0:64, bass.ts(cb, 512)],
                    rr[:],
                )
                # shifted duplicate: row 64+d col c = row d col c+1
                nc.sync.dma_start(
                    out=attnT2b[h][64:128, 512 * cb : 512 * cb + 511],
                    in_=attnT2b[h][0:64, 512 * cb + 1 : 512 * (cb + 1)],
                )

        def proj_qq(h, qq0, tail=False):
            """Project head h for quarter pair (qq0, qq0+1); one psum tile
            holds both quarters so the pss-tag rotation stalls half as often.
            In the tail, ys adds and y DMA issues are spread across engines
            so the final store ladder does not serialize on one queue."""
            a2 = attnT2b[h][:].rearrange("p (r s) -> p s r", s=16)
            psy = ps2.tile([128, 1024], F32, tag="pss", name="psy", bufs=3)
            for k in range(2):
                qq = qq0 + k
                if tail and k == 1:
                    # pre-add bo into psum so the psum->sbuf move can be an
                    # Act-engine Copy (Act is idle in the tail; DVE is not)
                    nc.tensor.matmul(
                        psy[:, bass.ts(k, 256)],
                        ones_col[:],
                        bo_sb[0:1, bass.ts(qq, 256)],
                        start=True,
                        stop=False,
                    )
                for mp in range(8):
                    nc.tensor.matmul(
                        psy[:, bass.ts(k, 256)],
                        a2[:, 2 * mp, :],
                        wo2_sb[:, mp, qq, :],
                        start=(mp == 0) and not (tail and k == 1),
                        stop=(mp == 7),
                    )
            for k in range(2):
                qq = qq0 + k
                ys = y_pool.tile([128, 256], F32, tag="ys", name="ys")
                if tail and k == 1:
                    nc.scalar.activation(
                        ys[:], psy[:, bass.ts(k, 256)],
                        mybir.ActivationFunctionType.Copy,
                    )
                else:
                    nc.vector.tensor_add(
                        ys[:], psy[:, bass.ts(k, 256)], bo_sb[:, bass.ts(qq, 256)]
                    )
                nc.sync.dma_start(out=y[bass.ts(h, 128), bass.ts(qq, 256)], in_=ys[:])

        qv0 = [(qst[0][0:64, :], kst[0][0:64, :]), (qtb[0][:], ktb[0][:])]
        qv1 = [(qst[1][0:64, :], kst[1][0:64, :]), (qtb[1][:], ktb[1][:])]

        def Q1(nb):
            return lambda: qk1_half(0, nb)

        def K1(nb):
            return lambda: qk1_half(1, nb)

        def N(p, cb):
            return lambda: norm_j(p, cb)

        def P(h, qq):
            return lambda: proj_qq(h, qq)

        # qk1 half-chunks (1.7us each) are deadline-scheduled: attn1-j needs
        # qst[1]/kst[1] block nb=j only. norm/proj of a pair fill the other
        # pair's attention. Slots stay under ~2us of PE work each.
        # Each j's psa copies (fin) are deferred into the next j, emitted
        # behind its first two scores so they never delay the Act feed.
        fin, _ = attn_j(0, 0, qv0, [Q1(0)])
        fin, _ = attn_j(0, 1, qv0, [K1(0)], pre=fin)
        fin, _ = attn_j(0, 2, qv0, [Q1(1)], [K1(1), N(0, 0)], pre=fin)
        fin, _ = attn_j(0, 3, qv0, [Q1(2)], [K1(2), N(0, 1)], pre=fin)

        fin, _ = attn_j(1, 0, qv1, [Q1(3)], [N(0, 2)], pre=fin)
        fin, _ = attn_j(1, 1, qv1, [K1(3)], [N(0, 3)], pre=fin)
        fin, _ = attn_j(1, 2, qv1, [P(0, 0)], [P(0, 2), N(1, 0)], pre=fin)
        fin, psa13 = attn_j(1, 3, qv1, [P(1, 0)], [P(1, 2), N(1, 1)], [N(1, 2)], pre=fin)

        # ramp keepers for attn1's Act-paced endgame (ready mid-attn1-j3)
        dmy2 = ps2.tile([128, 512], F32, tag="psa1", name="dmy2")
        for _ in range(24):
            nc.tensor.matmul(
                dmy2[0:2, :], oneh_sb[0:1, 0:2], dall[1][2][0][:],
                start=True, stop=True,
            )

        # tail: pair-1 last j norm (reciprocals read the psum denominator
        # rows directly, skipping the dall gather DMA), then remaining proj
        rd = [
            a_pool.tile([1, 512], BF16, tag=f"rd{q}", name=f"rd{q}") for q in range(2)
        ]
        with nc.allow_low_precision(reason="softmax denom reciprocal"):
            for q in range(2):
                nc.vector.reciprocal(rd[q][:], psa13[q][64:65, :])
        nc.vector.tensor_copy(attnT2b[2][0:65, bass.ts(3, 512)], psa13[0][0:65, :])
        nc.vector.tensor_copy(attnT2b[3][0:65, bass.ts(3, 512)], psa13[1][0:65, :])
        # ramp-keeper: harmless matmuls keep the PE p-state at full clock
        # while the tail norm chain (recip/mul/shift DMA) runs on DVE/Pool.
        # They read rd so they become ready exactly when the tail starts.
        dmy = ps2.tile([128, 512], F32, tag="psa0", name="dmy", bufs=1)
        for _ in range(12):
            nc.tensor.matmul(
                dmy[0:64, :], oneh_sb[0:1, 0:64], rd[0][:],
                start=True, stop=True,
            )
        for q in range(2):
            h = 2 + q
            rr = rr_pool.tile([64, 512], BF16, tag="rr", name="rrT")
            nc.gpsimd.partition_broadcast(rr[:], rd[q][:])
            nc.vector.tensor_mul(
                attnT2b[h][0:64, bass.ts(3, 512)],
                attnT2b[h][0:64, bass.ts(3, 512)],
                rr[:],
            )
            nc.sync.dma_start(
                out=attnT2b[h][64:128, 512 * 3 : 512 * 3 + 511],
                in_=attnT2b[h][0:64, 512 * 3 + 1 : 512 * 4],
            )
        proj_qq(2, 0, tail=True)
        proj_qq(2, 2, tail=True)
        proj_qq(3, 0, tail=True)
        for qq in (2, 3):
            a2 = attnT2b[3][:].rearrange("p (r s) -> p s r", s=16)
            psy = ps2.tile([128, 1024], F32, tag="pss", name="psy1", bufs=3)
            for mp in range(8):
                nc.tensor.matmul(
                    psy[:, 0:256],
                    a2[:, 2 * mp, :],
                    wo2_sb[:, mp, qq, :],
                    start=(mp == 0),
                    stop=(mp == 7),
                )
            ys = y_pool.tile([128, 256], F32, tag="ys", name="ys")
            eng_d = (nc.scalar, nc.sync)[qq - 2]
            nc.vector.tensor_add(ys[:], psy[:, 0:256], bo_sb[:, bass.ts(qq, 256)])
            eng_d.dma_start(out=y[bass.ts(3, 128), bass.ts(qq, 256)], in_=ys[:])

    nc.compile()
    return nc


def make_masks():
    kl = np.arange(128)[:, None]
    cl = np.arange(128)[None, :]
    tri = (kl <= cl).astype(NPBF16)  # [128 k, 128 c]
    return np.ascontiguousarray(np.stack([tri, tri], 1))  # [128, 2, 128]


def prep_core_inputs(c, x, Wq, Wk, Wv, Wo, bo):
    b, g = c // 4, c % 4
    heads = [4 * g + i for i in range(HPC)]
    xt = np.ascontiguousarray(
        x[b].T.reshape(CT, 128, S).transpose(1, 0, 2).astype(NPBF16)
    )

    def pack_pair(W, p):
        h0, h1 = heads[2 * p], heads[2 * p + 1]
        cols = np.concatenate(
            [W[:, 64 * h0 : 64 * h0 + 64], W[:, 64 * h1 : 64 * h1 + 64]], 1
        )
        return cols.reshape(CT, 128, 128)

    wq = np.ascontiguousarray(
        np.stack([pack_pair(Wq, p) for p in range(2)]).transpose(2, 0, 1, 3)
    ).astype(NPBF16)  # [128, 2, CT, 128]
    wk = np.ascontiguousarray(
        np.stack([pack_pair(Wk, p) for p in range(2)]).transpose(2, 0, 1, 3)
    ).astype(NPBF16)
    wv = np.ascontiguousarray(
        np.concatenate([Wv[:, 64 * h : 64 * h + 64] for h in heads], 1)
        .reshape(CT, 128, 256)
        .transpose(1, 0, 2)
    ).astype(NPBF16)  # [128, CT, 256]
    # wo2[d, mp, qq, :] = Wo[128*mp + d, 256*qq : 256*(qq+1)]
    wo2 = np.ascontiguousarray(
        Wo.reshape(8, 128, 4, 256).transpose(1, 0, 2, 3)
    ).astype(NPBF16)  # [128, 8, 4, 256]
    oneh = np.kron(np.eye(2, dtype=np.float32), np.ones((1, 64), np.float32)).astype(
        NPBF16
    )  # [2, 128]
    return {
        "xt": xt,
        "wq": wq,
        "wk": wk,
        "wv": wv,
        "wo2": wo2,
        "bo": bo.astype(NPBF16),
        "masks": make_masks(),
        "oneh": oneh,
    }


_NC_CACHE = []


def kernel(x, Wq, Wk, Wv, Wo, bo):
    from concourse import bass_utils

    x, Wq, Wk, Wv, Wo, bo = (
        np.asarray(x, np.float32),
        np.asarray(Wq, np.float32),
        np.asarray(Wk, np.float32),
        np.asarray(Wv, np.float32),
        np.asarray(Wo, np.float32),
        np.asarray(bo, np.float32),
    )
    if not _NC_CACHE:
        _NC_CACHE.append(build_nc())
    nc = _NC_CACHE[0]
    in_maps = [prep_core_inputs(c, x, Wq, Wk, Wv, Wo, bo) for c in range(NC)]
    res = bass_utils.run_bass_kernel_spmd(nc, in_maps, core_ids=list(range(NC)))
    out = np.empty((B, S, D), np.float32)
    for c in range(NC):
        b, g = c // 4, c % 4
        out[b, 512 * g : 512 * (g + 1), :] = res.results[c]["y"]
    return out


# revision 53
# speedup vs baseline: 1.0032x; 1.0032x over previous
"""Multi-head causal attention (B=2, S=2048, D=1024, H=16, HD=64) on 8 TRN2 cores.

Sharding: core c handles batch b = c//4 and heads 4*(c%4)..4*(c%4)+3.
The reference reshapes [b,h,s,hd] -> [b,s,1024] WITHOUT head transpose-back,
so output rows [128h, 128h+128) of y[b] depend only on head h: each core
produces a disjoint [512, 1024] block of the output. No collectives.

v3 (bf16 + trim + skewed pipeline + phase interleave):
  - All weight/activation DRAM inputs in bf16 (host-converted).
  - Diagonal score tiles trimmed: scores matmul, exp, and PV only cover
    q-cols >= 128r of the 512-q block; the partial 128x128 triangle block is
    masked post-exp by one strided bf16 DVE multiply (both heads at once).
  - Attention inner loop is software-pipelined: scores(t+1) is emitted
    before PV(t) so the in-order PE queue never serializes behind exp(t).
  - Pair-1 Q/K projections and pair-0 normalize/projection are emitted as
    fillers inside the opposite pair's attention j-loop, placed right after
    the second scores emission (where PE would otherwise stall on exp).
  - Normalization is per-j: denominators ride the PV matmul as a 65th V
    column, are DMA-gathered into dall rows (2j+q), reciprocal'd per j-block,
    broadcast via one-hot matmul, applied by DVE/Pool multiplies.
  - Output projection at K=128: attnT2b[h] is [128, 2048] with partitions
    64:128 holding a 1-col-left-shifted copy of rows 0:64 (SBUF->SBUF DMA),
    so lhsT [128,128] packs head-chunk pairs (m, m+1) and Wo contracts in 8
    chunks of 128 instead of 16 of 64 (halves proj PE rows).
"""

import sys

if "/opt/trn_rl_repo" not in sys.path:
    sys.path.insert(0, "/opt/trn_rl_repo")

from contextlib import ExitStack

import numpy as np
import ml_dtypes

import concourse.bass as bass
import concourse.tile as tile
from concourse import bacc, mybir

F32 = mybir.dt.float32
F32R = mybir.dt.float32r
BF16 = mybir.dt.bfloat16
EXP = mybir.ActivationFunctionType.Exp

B, S, D, H, HD = 2, 2048, 1024, 16, 64
NC = 8
HPC = 4  # heads per core
CT = D // 128  # 8 contraction tiles
QB = 4  # q-blocks of 512
KT = S // 128  # 16 k-tiles
SCALE = 1.0 / 8.0
NPBF16 = ml_dtypes.bfloat16


def build_nc():
    nc = bacc.Bacc("TRN2", target_bir_lowering=False, debug=False)

    xt = nc.dram_tensor("xt", [128, CT, S], BF16, kind="ExternalInput").ap()
    wq = nc.dram_tensor("wq", [128, 2, CT, 128], BF16, kind="ExternalInput").ap()
    wk = nc.dram_tensor("wk", [128, 2, CT, 128], BF16, kind="ExternalInput").ap()
    wv = nc.dram_tensor("wv", [128, CT, 256], BF16, kind="ExternalInput").ap()
    wo2 = nc.dram_tensor("wo2", [128, 8, 4, 256], BF16, kind="ExternalInput").ap()
    bo = nc.dram_tensor("bo", [D], BF16, kind="ExternalInput").ap()
    masks = nc.dram_tensor("masks", [128, 2, 128], BF16, kind="ExternalInput").ap()
    oneh = nc.dram_tensor("oneh", [2, 128], BF16, kind="ExternalInput").ap()
    y = nc.dram_tensor("y", [HPC * 128, D], F32, kind="ExternalOutput").ap()

    with tile.TileContext(nc) as tc, ExitStack() as ctx:
        a_pool = ctx.enter_context(tc.tile_pool(name="a", bufs=1))

        # ---- resident SBUF tensors
        xt_sb = a_pool.tile([128, CT, S], BF16, tag="xt")
        wq_sb = a_pool.tile([128, 2, CT, 128], BF16, tag="wq")
        wk_sb = a_pool.tile([128, 2, CT, 128], BF16, tag="wk")
        wv_sb = a_pool.tile([128, CT, 256], BF16, tag="wv")
        wo2_sb = a_pool.tile([128, 8, 4, 256], BF16, tag="wo2")
        masks_sb = a_pool.tile([128, 2, 128], BF16, tag="masks")
        oneh_sb = a_pool.tile([2, 128], BF16, tag="oneh")
        bo_sb = a_pool.tile([128, D], BF16, tag="bo")
        # V packed [128(s_local), 16 s-tiles, 4*(64+ones col)] bf16
        v4 = a_pool.tile([128, KT, 260], BF16, tag="v4")
        qst = [a_pool.tile([128, S], BF16, tag=f"qst{p}", name=f"qst{p}") for p in range(2)]
        kst = [a_pool.tile([128, S], BF16, tag=f"kst{p}", name=f"kst{p}") for p in range(2)]
        qtb = [a_pool.tile([64, S], BF16, tag=f"qtb{p}", name=f"qtb{p}") for p in range(2)]
        ktb = [a_pool.tile([64, S], BF16, tag=f"ktb{p}", name=f"ktb{p}") for p in range(2)]
        # attnT2b[h]: rows 0:64 = attn^T (hd x q), rows 64:128 = 1-col-left-
        # shifted copy (for K=128 proj lhsT)
        attnT2b = [
            a_pool.tile([128, S], BF16, tag=f"at{h}", name=f"at{h}") for h in range(HPC)
        ]
        # dall[p][j][q] = denominators of q-block j, head q of pair p
        # (separate [1,512] tiles: partition_broadcast needs base partition 0)
        dall = [
            [[a_pool.tile([1, 512], BF16, tag=f"dall{p}{j}{q}", name=f"dall{p}{j}{q}")
              for q in range(2)] for j in range(QB)]
            for p in range(2)
        ]
        dallr = [
            [[a_pool.tile([1, 512], BF16, tag=f"dallr{p}{j}{q}", name=f"dallr{p}{j}{q}")
              for q in range(2)] for j in range(QB)]
            for p in range(2)
        ]

        # warm up the Act engine's Exp table at t~0 (it otherwise lazy-loads
        # 1.3us right at the first attention exp)
        ones_col = a_pool.tile([1, 128], BF16, tag="ones_col")
        nc.vector.memset(ones_col[:], 1.0)
        warm = a_pool.tile([1, 8], F32, tag="warm")
        warm2 = a_pool.tile([1, 8], F32, tag="warm2")
        nc.vector.memset(warm[:], 0.0)
        nc.scalar.activation(warm2[:], warm[:], EXP, scale=SCALE)

        # ---- input DMAs (SP queue; order = need order; wo2 issued after P1)
        nc.sync.dma_start(out=wq_sb[:, 0, 0], in_=wq[:, 0, 0])
        nc.sync.dma_start(out=wk_sb[:, 0, 0], in_=wk[:, 0, 0])
        nc.sync.dma_start(out=xt_sb[:, 0, :], in_=xt[:, 0, :])
        nc.sync.dma_start(out=wq_sb[:, 0, 1:8], in_=wq[:, 0, 1:8])
        nc.sync.dma_start(out=wk_sb[:, 0, 1:8], in_=wk[:, 0, 1:8])
        nc.sync.dma_start(out=xt_sb[:, 1, :], in_=xt[:, 1, :])
        for quad in range(1, 4):
            nc.sync.dma_start(
                out=xt_sb[:, 2 * quad : 2 * quad + 2, :],
                in_=xt[:, 2 * quad : 2 * quad + 2, :],
            )
        nc.sync.dma_start(out=wv_sb[:], in_=wv)
        nc.sync.dma_start(out=wq_sb[:, 1], in_=wq[:, 1])
        nc.sync.dma_start(out=wk_sb[:, 1], in_=wk[:, 1])
        nc.sync.dma_start(out=masks_sb[:], in_=masks)
        nc.sync.dma_start(out=oneh_sb[:], in_=oneh)
        bo_b = bass.AP(tensor=bo.tensor, offset=bo.offset, ap=[[0, 128], [1, D]])
        nc.sync.dma_start(out=bo_sb[:], in_=bo_b)
        # ones column of v4 via memset (strided view)
        nc.gpsimd.memset(
            v4[:].rearrange("p t (h c) -> p t h c", c=65)[:, :, :, 64:65], 1.0
        )

        y_pool = ctx.enter_context(tc.tile_pool(name="y", bufs=6))
        rr_pool = ctx.enter_context(tc.tile_pool(name="rr", bufs=8))
        pt_pool = ctx.enter_context(tc.tile_pool(name="pt", bufs=3))

        # ---- P1 pair 0: Q/K ct-outer with 8 live psum accumulators, then V
        with ExitStack() as scope1:
            ps1 = scope1.enter_context(tc.tile_pool(name="ps1", bufs=2, space="PSUM"))
            psqk = [
                ps1.tile([128, 512], F32, tag=f"qk{i}", name=f"qk{i}", bufs=1)
                for i in range(8)
            ]
            for ct in range(CT):
                for i, w_sb in ((0, wq_sb), (4, wk_sb)):
                    for nb in range(QB):
                        nc.tensor.matmul(
                            psqk[i + nb][:],
                            w_sb[:, 0, ct, :],
                            xt_sb[:, ct, bass.ts(nb, 512)],
                            start=(ct == 0),
                            stop=(ct == CT - 1),
                        )
            for i, dst in ((0, qst[0]), (4, kst[0])):
                for nb in range(QB):
                    nc.vector.tensor_copy(dst[:, bass.ts(nb, 512)], psqk[i + nb][:])
            nc.sync.dma_start(out=qtb[0][:], in_=qst[0][64:128, :])
            nc.sync.dma_start(out=ktb[0][:], in_=kst[0][64:128, :])
            nc.sync.dma_start(out=wo2_sb[:], in_=wo2)
            # V for all 4 heads (st-outer, ct accumulation)
            for st in range(KT):
                ps = ps1.tile([128, 256], F32, tag=f"qk{st % 8}", name="psv", bufs=1)
                for ct in range(CT):
                    nc.tensor.matmul(
                        ps[:],
                        xt_sb[:, ct, bass.ts(st, 128)],
                        wv_sb[:, ct, :],
                        start=(ct == 0),
                        stop=(ct == CT - 1),
                    )
                nc.scalar.activation(
                    v4[:, st, :].rearrange("p (h c) -> p h c", c=65)[:, :, 0:64],
                    ps[:].rearrange("p (h c) -> p h c", c=64),
                    mybir.ActivationFunctionType.Copy,
                )

        # ---- P2: attention, software-pipelined, with interleaved fillers
        ps2 = ctx.enter_context(tc.tile_pool(name="ps2", bufs=2, space="PSUM"))

        def attn_scores(p, j, t, qv):
            """scores^T both heads -> exp -> mask (diagonal). Returns pt2."""
            r = t - 4 * j
            c0 = 128 * r if r > 0 else 0
            pss = ps2.tile([128, 1024], F32, tag="pss", name="pss", bufs=3)
            for q in range(2):
                qt, kt = qv[q]
                nc.tensor.matmul(
                    pss[:, 512 * q + c0 : 512 * (q + 1)],
                    kt[:, bass.ts(t, 128)],
                    qt[:, 512 * j + c0 : 512 * (j + 1)],
                    start=True,
                    stop=True,
                )
            pt2 = pt_pool.tile([128, 1024], BF16, tag="pt2", name="pt2")
            if r < 0:
                nc.scalar.activation(pt2[:], pss[:], EXP, scale=SCALE)
            else:
                pv = pss[:].rearrange("p (h c) -> p h c", c=512)[:, :, c0:512]
                ov = pt2[:].rearrange("p (h c) -> p h c", c=512)[:, :, c0:512]
                nc.scalar.activation(ov, pv, EXP, scale=SCALE)
                mv = pt2[:].rearrange("p (h c) -> p h c", c=512)[:, :, c0 : c0 + 128]
                nc.vector.tensor_mul(mv, mv, masks_sb[:])
            return pt2

        def attn_pv(p, j, t, pt2, psa):
            r = t - 4 * j
            c0 = 128 * r if r > 0 else 0
            last = t == 4 * j + 3
            for q in range(2):
                h = 2 * p + q
                vsl = v4[:, t, bass.ds(65 * h, 65)]
                if r >= 0:
                    # split stop: [c0, c0+128) is final here; rest continues
                    nc.tensor.matmul(
                        psa[q][0:65, c0 : c0 + 128],
                        vsl,
                        pt2[:, 512 * q + c0 : 512 * q + c0 + 128],
                        start=(t == 0),
                        stop=True,
                    )
                    if not last:
                        nc.tensor.matmul(
                            psa[q][0:65, c0 + 128 : 512],
                            vsl,
                            pt2[:, 512 * q + c0 + 128 : 512 * (q + 1)],
                            start=(t == 0),
                            stop=False,
                        )
                else:
                    nc.tensor.matmul(
                        psa[q][0:65, :],
                        vsl,
                        pt2[:, bass.ts(q, 512)],
                        start=(t == 0),
                        stop=False,
                    )

        def attn_j(p, j, qv, slot0=(), slot1=(), slot2=(), pre=None):
            """Skewed pipeline: S0 S1 [pre] [slot0] P0 S2 [slot1] P1 S3 ... PT.
            Each slot must stay under ~2us of PE work (the Act runway from
            the two queued exps) or Act starves behind the in-order PE queue.
            Returns a closure finishing this j (psa copies + denom gather),
            which the caller passes as `pre` to the NEXT attn_j so the next
            block's first scores are not queued behind it."""
            T = 4 * j + 4
            psa = [
                ps2.tile([128, 512], F32, tag=f"psa{q}", name=f"psa{q}", bufs=1)
                for q in range(2)
            ]
            pts = [attn_scores(p, j, 0, qv)]
            if T > 1:
                pts.append(attn_scores(p, j, 1, qv))
            if pre is not None:
                pre()
            for f in slot0:
                f()
            for t in range(T):
                if t + 2 < T:
                    pts.append(attn_scores(p, j, t + 2, qv))
                if t == 1:
                    for f in slot1:
                        f()
                if t == 5:
                    for f in slot2:
                        f()
                attn_pv(p, j, t, pts[t], psa)

            def finish():
                for q in range(2):
                    h = 2 * p + q
                    for half in range(2):
                        nc.vector.tensor_copy(
                            attnT2b[h][0:65, 512 * j + 256 * half : 512 * j + 256 * (half + 1)],
                            psa[q][0:65, bass.ts(half, 256)],
                        )
                    nc.sync.dma_start(
                        out=dall[p][j][q][:],
                        in_=attnT2b[h][64:65, bass.ts(j, 512)],
                    )

            return finish, psa

        def qk1_half(half, nb):
            """Pair-1 Q (half=0) or K (half=1) projection for one 512-col
            s-block (rides pss tag; ~1.7us of PE)."""
            w_sb, dst, dstb = ((wq_sb, qst[1], qtb[1]), (wk_sb, kst[1], ktb[1]))[half]
            pssqk = ps2.tile([128, 1024], F32, tag="pss", name="pssqk", bufs=3)
            for ct in range(CT):
                nc.tensor.matmul(
                    pssqk[:, 0:512],
                    w_sb[:, 1, ct, :],
                    xt_sb[:, ct, bass.ts(nb, 512)],
                    start=(ct == 0),
                    stop=(ct == CT - 1),
                )
            nc.vector.tensor_copy(dst[:, bass.ts(nb, 512)], pssqk[:, 0:512])
            nc.sync.dma_start(
                out=dstb[:, bass.ts(nb, 512)], in_=dst[64:128, bass.ts(nb, 512)]
            )

        def norm_j(p, cb):
            """Normalize q-block cb of both heads of pair p, then emit the
            shifted duplicate rows for the K=128 projection. The reciprocal
            rows are partition-broadcast by DMA (stride-0 source) so the
            multiplies are pure-SBUF bf16 (DVE 4x) and PE stays free."""
            for q in range(2):
                h = 2 * p + q
                with nc.allow_low_precision(reason="softmax denom reciprocal"):
                    nc.vector.reciprocal(dallr[p][cb][q][:], dall[p][cb][q][:])
                rr = rr_pool.tile([64, 512], BF16, tag="rr", name="rr")
                nc.gpsimd.partition_broadcast(rr[:], dallr[p][cb][q][:])
                nc.vector.tensor_mul(
                    attnT2b[h][0:64, bass.ts(cb, 512)],
                    attnT2b[h][0:64, bass.ts(cb, 512)],
                    rr[:],
                )
                # shifted duplicate: row 64+d col c = row d col c+1
                nc.sync.dma_start(
                    out=attnT2b[h][64:128, 512 * cb : 512 * cb + 511],
                    in_=attnT2b[h][0:64, 512 * cb + 1 : 512 * (cb + 1)],
                )

        def proj_qq(h, qq0, tail=False):
            """Project head h for quarter pair (qq0, qq0+1); one psum tile
            holds both quarters so the pss-tag rotation stalls half as often.
            In the tail, ys adds and y DMA issues are spread across engines
            so the final store ladder does not serialize on one queue."""
            a2 = attnT2b[h][:].rearrange("p (r s) -> p s r", s=16)
            psy = ps2.tile([128, 1024], F32, tag="pss", name="psy", bufs=3)
            for k in range(2):
                qq = qq0 + k
                if tail and k == 1:
                    # pre-add bo into psum so the psum->sbuf move can be an
                    # Act-engine Copy (Act is idle in the tail; DVE is not)
                    nc.tensor.matmul(
                        psy[:, bass.ts(k, 256)],
                        ones_col[:],
                        bo_sb[0:1, bass.ts(qq, 256)],
                        start=True,
                        stop=False,
                    )
                for mp in range(8):
                    nc.tensor.matmul(
                        psy[:, bass.ts(k, 256)],
                        a2[:, 2 * mp, :],
                        wo2_sb[:, mp, qq, :],
                        start=(mp == 0) and not (tail and k == 1),
                        stop=(mp == 7),
                    )
            for k in range(2):
                qq = qq0 + k
                ys = y_pool.tile([128, 256], F32, tag="ys", name="ys")
                if tail and k == 1:
                    nc.scalar.activation(
                        ys[:], psy[:, bass.ts(k, 256)],
                        mybir.ActivationFunctionType.Copy,
                    )
                else:
                    nc.vector.tensor_add(
                        ys[:], psy[:, bass.ts(k, 256)], bo_sb[:, bass.ts(qq, 256)]
                    )
                nc.sync.dma_start(out=y[bass.ts(h, 128), bass.ts(qq, 256)], in_=ys[:])

        qv0 = [(qst[0][0:64, :], kst[0][0:64, :]), (qtb[0][:], ktb[0][:])]
        qv1 = [(qst[1][0:64, :], kst[1][0:64, :]), (qtb[1][:], ktb[1][:])]

        def Q1(nb):
            return lambda: qk1_half(0, nb)

        def K1(nb):
            return lambda: qk1_half(1, nb)

        def N(p, cb):
            return lambda: norm_j(p, cb)

        def P(h, qq):
            return lambda: proj_qq(h, qq)

        # qk1 half-chunks (1.7us each) are deadline-scheduled: attn1-j needs
        # qst[1]/kst[1] block nb=j only. norm/proj of a pair fill the other
        # pair's attention. Slots stay under ~2us of PE work each.
        # Each j's psa copies (fin) are deferred into the next j, emitted
        # behind its first two scores so they never delay the Act feed.
        fin, _ = attn_j(0, 0, qv0, [Q1(0)])
        fin, _ = attn_j(0, 1, qv0, [K1(0)], pre=fin)
        fin, _ = attn_j(0, 2, qv0, [Q1(1)], [K1(1), N(0, 0)], pre=fin)
        fin, _ = attn_j(0, 3, qv0, [Q1(2)], [K1(2), N(0, 1)], pre=fin)

        fin, _ = attn_j(1, 0, qv1, [Q1(3)], [N(0, 2)], pre=fin)
        fin, _ = attn_j(1, 1, qv1, [K1(3)], [N(0, 3)], pre=fin)
        fin, _ = attn_j(1, 2, qv1, [P(0, 0)], [P(0, 2), N(1, 0)], pre=fin)
        fin, psa13 = attn_j(1, 3, qv1, [P(1, 0)], [P(1, 2), N(1, 1)], [N(1, 2)], pre=fin)

        # ramp keepers for attn1's Act-paced endgame (ready mid-attn1-j3)
        dmy2 = ps2.tile([128, 512], F32, tag="psa1", name="dmy2")
        for _ in range(24):
            nc.tensor.matmul(
                dmy2[0:2, :], oneh_sb[0:1, 0:2], dall[1][2][0][:],
                start=True, stop=True,
            )

        # tail: pair-1 last j norm (reciprocals read the psum denominator
        # rows directly, skipping the dall gather DMA), then remaining proj
        rd = [
            a_pool.tile([1, 512], BF16, tag=f"rd{q}", name=f"rd{q}") for q in range(2)
        ]
        with nc.allow_low_precision(reason="softmax denom reciprocal"):
            for q in range(2):
                nc.vector.reciprocal(rd[q][:], psa13[q][64:65, :])
        nc.vector.tensor_copy(attnT2b[2][0:65, bass.ts(3, 512)], psa13[0][0:65, :])
        nc.vector.tensor_copy(attnT2b[3][0:65, bass.ts(3, 512)], psa13[1][0:65, :])
        # ramp-keeper: harmless matmuls keep the PE p-state at full clock
        # while the tail norm chain (recip/mul/shift DMA) runs on DVE/Pool.
        # They read rd so they become ready exactly when the tail starts.
        dmy = ps2.tile([128, 512], F32, tag="psa0", name="dmy", bufs=1)
        for _ in range(12):
            nc.tensor.matmul(
                dmy[0:64, :], oneh_sb[0:1, 0:64], rd[0][:],
                start=True, stop=True,
            )
        for q in range(2):
            h = 2 + q
            rr = rr_pool.tile([64, 512], BF16, tag="rr", name="rrT")
            nc.gpsimd.partition_broadcast(rr[:], rd[q][:])
            nc.vector.tensor_mul(
                attnT2b[h][0:64, bass.ts(3, 512)],
                attnT2b[h][0:64, bass.ts(3, 512)],
                rr[:],
            )
            nc.sync.dma_start(
                out=attnT2b[h][64:128, 512 * 3 : 512 * 3 + 511],
                in_=attnT2b[h][0:64, 512 * 3 + 1 : 512 * 4],
            )
        proj_qq(2, 0, tail=True)
        proj_qq(2, 2, tail=True)
        proj_qq(3, 0, tail=True)
        for qq in (2, 3):
            a2 = attnT2b[3][:].rearrange("p (r s) -> p s r", s=16)
            psy = ps2.tile([128, 1024], F32, tag="pss", name="psy1", bufs=3)
            for mp in range(8):
                nc.tensor.matmul(
                    psy[:, 0:256],
                    a2[:, 2 * mp, :],
                    wo2_sb[:, mp, qq, :],
                    start=(mp == 0),
                    stop=(mp == 7),
                )
            ys = y_pool.tile([128, 256], F32, tag="ys", name="ys")
            eng_d = (nc.scalar, nc.sync)[qq - 2]
            nc.vector.tensor_add(ys[:], psy[:, 0:256], bo_sb[:, bass.ts(qq, 256)])
            eng_d.dma_start(out=y[bass.ts(3, 128), bass.ts(qq, 256)], in_=ys[:])

    nc.compile()
    return nc


def make_masks():
    kl = np.arange(128)[:, None]
    cl = np.arange(128)[None, :]
    tri = (kl <= cl).astype(NPBF16)  # [128 k, 128 c]
    return np.ascontiguousarray(np.stack([tri, tri], 1))  # [128, 2, 128]


def prep_core_inputs(c, x, Wq, Wk, Wv, Wo, bo):
    b, g = c // 4, c % 4
    heads = [4 * g + i for i in range(HPC)]
    xt = np.ascontiguousarray(
        x[b].T.reshape(CT, 128, S).transpose(1, 0, 2).astype(NPBF16)
    )

    def pack_pair(W, p):
        h0, h1 = heads[2 * p], heads[2 * p + 1]
        cols = np.concatenate(
            [W[:, 64 * h0 : 64 * h0 + 64], W[:, 64 * h1 : 64 * h1 + 64]], 1
        )
        return cols.reshape(CT, 128, 128)

    wq = np.ascontiguousarray(
        np.stack([pack_pair(Wq, p) for p in range(2)]).transpose(2, 0, 1, 3)
    ).astype(NPBF16)  # [128, 2, CT, 128]
    wk = np.ascontiguousarray(
        np.stack([pack_pair(Wk, p) for p in range(2)]).transpose(2, 0, 1, 3)
    ).astype(NPBF16)
    wv = np.ascontiguousarray(
        np.concatenate([Wv[:, 64 * h : 64 * h + 64] for h in heads], 1)
        .reshape(CT, 128, 256)
        .transpose(1, 0, 2)
    ).astype(NPBF16)  # [128, CT, 256]
    # wo2[d, mp, qq, :] = Wo[128*mp + d, 256*qq : 256*(qq+1)]
    wo2 = np.ascontiguousarray(
        Wo.reshape(8, 128, 4, 256).transpose(1, 0, 2, 3)
    ).astype(NPBF16)  # [128, 8, 4, 256]
    oneh = np.kron(np.eye(2, dtype=np.float32), np.ones((1, 64), np.float32)).astype(
        NPBF16
    )  # [2, 128]
    return {
        "xt": xt,
        "wq": wq,
        "wk": wk,
        "wv": wv,
        "wo2": wo2,
        "bo": bo.astype(NPBF16),
        "masks": make_masks(),
        "oneh": oneh,
    }


_NC_CACHE = []


def kernel(x, Wq, Wk, Wv, Wo, bo):
    from concourse import bass_utils

    x, Wq, Wk, Wv, Wo, bo = (
        np.asarray(x, np.float32),
        np.asarray(Wq, np.float32),
        np.asarray(Wk, np.float32),
        np.asarray(Wv, np.float32),
        np.asarray(Wo, np.float32),
        np.asarray(bo, np.float32),
    )
    if not _NC_CACHE:
        _NC_CACHE.append(build_nc())
    nc = _NC_CACHE[0]
    in_maps = [prep_core_inputs(c, x, Wq, Wk, Wv, Wo, bo) for c in range(NC)]
    res = bass_utils.run_bass_kernel_spmd(nc, in_maps, core_ids=list(range(NC)))
    out = np.empty((B, S, D), np.float32)
    for c in range(NC):
        b, g = c // 4, c % 4
        out[b, 512 * g : 512 * (g + 1), :] = res.results[c]["y"]
    return out
